# revision 60
# baseline (speedup 1.0000x reference)
"""CeptaTransformerBlock Trainium2 kernel.

Sharding: data-parallel over the 4096 (B*T) rows -> 8 cores x 512 contiguous
rows (cores 0-3: batch 0, cores 4-7: batch 1). The SSM scan is computed
locally per core with tensor_tensor_scan; the cross-core carry state is
reconstructed from per-core (end-state, decay-product) pairs exchanged with
one tiny AllGather (4KB/core).

Precision plan (rel err ~9e-3 vs the 2e-2 gate):
- context path (to_P / emb / F-gate / low-rank y): bf16 operands at full
  PE rate; the scan itself stays fp32 on the DVE.
- 16x MLP: fp8e4 DoubleRow matmuls (K=256 per instruction, half-cycle
  per psum row) with power-of-two weight scales (16x on fc1, 64x on fc2)
  undone in the gelu pre-scale and the psum eviction.
- the y path uses the low-rank factorization explicitly:
  y = t @ w_fromP + ((h @ b_r) @ (c_r @ w_fromP)), with the cross-core
  carry folded into the tiny b_r stationary operand (hin * b_r).
RMSNorm weight vectors are folded into the following matmul weights on
the host; 1/rms comes from a DVE bit-trick rsqrt for norm2 (no ACT table
swaps mid-kernel) and is applied as a per-partition ACT scale.
"""

from contextlib import ExitStack

import numpy as np

import concourse.bass as bass
import concourse.tile as tile
from concourse import bacc, mybir
from concourse.bass_utils import run_bass_kernel_spmd
from concourse.masks import make_identity

F32 = mybir.dt.float32
F32R = mybir.dt.float32r
BF16 = mybir.dt.bfloat16
FP8 = mybir.dt.float8e4
DR = mybir.MatmulPerfMode.DoubleRow
AF = mybir.ActivationFunctionType
OP = mybir.AluOpType

B, T, D = 2, 2048, 1024
P, ALPHA, PR = 512, 4, 64
HID = 16 * D
EPS = 1e-6

N_CORES = 8
TLOC = (B * T) // N_CORES       # 512 rows per core
NRB = TLOC // 128               # 4 row blocks
NDK = D // 128                  # 8 contraction subtiles over D
NKB = D // 256                  # 4 DoubleRow contraction subtiles over D
NPM = P // 128                  # 4 P-channel subtiles
HK_TILE = 1024                  # MLP hidden tile
NHK = HID // HK_TILE            # 16
NHM = HK_TILE // 128            # 8
W1S = 16.0                      # fp8 scale on w_fc1
W2S = 64.0                      # fp8 scale on w_fc2


def _build_nc(collective=True):
    nc = bacc.Bacc("TRN2", target_bir_lowering=False, debug=False,
                   num_devices=N_CORES)

    x_loc = nc.dram_tensor("x_loc", [TLOC, D], BF16, kind="ExternalInput").ap()
    hsel = nc.dram_tensor("hsel", [128, 8], F32, kind="ExternalInput").ap()
    wtoP_t = nc.dram_tensor("wtoP_t", [128, NDK, P], BF16, kind="ExternalInput").ap()
    wf_t = nc.dram_tensor("wf_t", [128, NDK, P], BF16, kind="ExternalInput").ap()
    wemb_t = nc.dram_tensor("wemb_t", [ALPHA, NPM, 128, NDK, 128], BF16,
                            kind="ExternalInput").ap()
    wfromP_t = nc.dram_tensor("wfromP_t", [NPM, 128, D], BF16,
                              kind="ExternalInput").ap()
    br_t = nc.dram_tensor("br_t", [NPM, 128, PR], BF16,
                          kind="ExternalInput").ap()
    W2_t = nc.dram_tensor("W2_t", [PR, D], BF16, kind="ExternalInput").ap()
    # fp8 MLP weights: per-hk chunks, contiguous per partition.
    # wfc1_8[hk, p, o, kb, i, m] = 16 * w_fc1[kb*256+i*128+p, (hk*8+o)*128+m]
    wfc1_8 = nc.dram_tensor("wfc1_8", [NHK, 128, 8, NKB, 2, 128], FP8,
                            kind="ExternalInput").ap()
    # wfc2_8[hk, p, h, i, d] = 64 * w_fc2[((hk*4+h)*2+i)*128+p, d]
    wfc2_8 = nc.dram_tensor("wfc2_8", [NHK, 128, 4, 2, D], FP8,
                            kind="ExternalInput").ap()

    out_loc = nc.dram_tensor("out_loc", [TLOC, D], F32, kind="ExternalOutput").ap()

    with tile.TileContext(nc) as tc:
        _emit(nc, tc, x_loc, hsel, wtoP_t, wf_t, wemb_t, wfromP_t,
              br_t, W2_t, wfc1_8, wfc2_8, out_loc, collective)
    nc.compile()
    return nc


def _emit(nc, tc, x_loc, hsel, wtoP_t, wf_t, wemb_t, wfromP_t,
          br_t, W2_t, wfc1_8, wfc2_8, out_loc, collective=True):
    ctx = ExitStack()
    with ctx:
        const = ctx.enter_context(tc.tile_pool(name="const", bufs=1))
        persist = ctx.enter_context(tc.tile_pool(name="persist", bufs=1))

        ident = const.tile([128, 128], F32)
        make_identity(nc, ident[:])
        ident_b = const.tile([128, 128], BF16)
        make_identity(nc, ident_b[:])

        # PE warm-up: ~3.5us of throwaway matmuls while the x DMA is in
        # flight gets the p-state ramp done (and the head gap filled) before
        # the first real transpose issues
        with tc.tile_pool(name="warm_ps", bufs=1, space="PSUM") as warm_ps:
            wp = warm_ps.tile([128, 512], F32)
            wsrc = const.tile([128, 512], BF16)
            nc.vector.memset(wsrc[:], 0.0)
            for _ in range(9):
                nc.tensor.matmul(wp[:], ident_b[:], wsrc[:],
                                 start=True, stop=True)
        epst = const.tile([128, 1], F32)
        nc.vector.memset(epst[:], EPS)
        hsel_t = const.tile([128, 8], F32)
        with tc.tile_wait_until(0.03):
            nc.sync.dma_start(hsel_t[:], hsel[:])
        ones_row = const.tile([1, 128], F32)
        nc.vector.memset(ones_row[:], 1.0)
        # warm the ACT Sqrt table during the initial x DMA: the first real
        # Sqrt in norm1 then skips its ~1.3us cold table load
        warm = const.tile([128, 1], F32)
        nc.scalar.activation(warm[:], epst[:], AF.Sqrt)
        magic_i = const.tile([128, 1], mybir.dt.int32)
        nc.vector.memset(magic_i[:], 0x5F3759DF)

        x2_rows = [persist.tile([128, D], F32, name=f"x2r{rb}")
                   for rb in range(NRB)]   # x + y (residual mid)

        # mm1 weight stream: opened at top level so chunks prefetch on the
        # idle Pool DMA queue during the context phase
        wfc1_pool = ctx.enter_context(tc.tile_pool(name="wfc1_pool", bufs=4))
        w1cs = {}

        def prefetch_w1(hk, at=None):
            # `at` (ms virtual time) stages dep-free chunk DMAs so they use
            # the exchange-window bus idle instead of racing the context's
            # own DMAs at t=0 (the Tile scheduler is ready-first)
            with tc.tile_wait_until(at if at is not None else 0,
                                    enable=at is not None):
                w1c = wfc1_pool.tile([128, 8, NKB, 2, 128], FP8, name="w1c")
                nc.gpsimd.dma_start(w1c[:], wfc1_8[hk])
            w1cs[hk] = w1c

        def rmsnorm_transpose(src_rows, dst_T, tag, act_stats=False,
                              tr_rows=None, tr_dtype=F32, tr_ident=None):
            """src_rows: NRB tiles [128, D] f32 row-major -> dst_T [128, NDK, TLOC]
            f32r/fp8 feature-major, scaled by 1/rms(row).

            act_stats=True computes the row stats with ACT Square+accum and
            1/sqrt with a DVE bit-trick rsqrt (no ACT table residency
            change: Square/Copy live in every set), so the gelu table stays
            loaded across norm2 and DVE keeps its slack for evictions."""
            with tc.tile_pool(name=f"nrm_{tag}", bufs=2) as np_, \
                 tc.tile_pool(name=f"nrmc_{tag}", bufs=4) as npc, \
                 tc.tile_pool(name=f"ptr_{tag}", bufs=6, space="PSUM") as ptr_ps:
                scales = []
                rscs = []
                if act_stats:
                    # stats split DVE/ACT(Square) + paired DVE bit-trick
                    # rsqrt: norm2 touches NO ACT table function, so the
                    # gelu set stays resident from the context through mm1
                    I32 = mybir.dt.int32
                    ms4 = npc.tile([128, NRB], F32, name=f"ms_{tag}")
                    for pair in range(2):
                        for j in range(2):
                            rb = pair * 2 + j
                            sq = np_.tile([128, D], src_rows[rb].dtype,
                                          name=f"sq_{tag}")
                            if j == 0:
                                nc.vector.scalar_tensor_tensor(
                                    sq[:], src_rows[rb][:], 1.0,
                                    src_rows[rb][:], OP.mult, OP.mult,
                                    accum_out=ms4[:, rb:rb + 1])
                            else:
                                nc.scalar.activation(
                                    sq[:], src_rows[rb][:], AF.Square,
                                    accum_out=ms4[:, rb:rb + 1])
                        sl = ms4[:, pair * 2:pair * 2 + 2]
                        msn = npc.tile([128, 2], F32, name=f"msn{pair}_{tag}")
                        nc.vector.tensor_scalar(msn[:], sl, 1.0 / D, EPS,
                                                OP.mult, OP.add)
                        ti = npc.tile([128, 2], I32, name=f"ti{pair}_{tag}")
                        nc.vector.tensor_scalar(ti[:], msn[:].bitcast(I32), 1,
                                                None, OP.logical_shift_right)
                        yi = npc.tile([128, 2], I32, name=f"yi{pair}_{tag}")
                        nc.vector.tensor_tensor(
                            yi[:], magic_i[:].broadcast_to([128, 2]), ti[:],
                            OP.subtract)
                        cur = yi[:].bitcast(F32)
                        # one Newton iteration: ~0.18% scale error, far
                        # below the fp8 noise downstream
                        for it in range(1):
                            a = npc.tile([128, 2], F32,
                                         name=f"a{it}{pair}_{tag}")
                            nc.vector.tensor_tensor(a[:], cur, cur, OP.mult)
                            nc.vector.tensor_tensor(a[:], a[:], msn[:],
                                                    OP.mult)
                            nc.vector.tensor_scalar(a[:], a[:], -0.5, 1.5,
                                                    OP.mult, OP.add)
                            nxt = npc.tile([128, 2], F32,
                                           name=f"y{it}{pair}_{tag}")
                            nc.vector.tensor_tensor(nxt[:], cur, a[:],
                                                    OP.mult)
                            cur = nxt[:]
                        rscs.append(cur)
                    rscs = [rscs[rb // 2][:, rb % 2:rb % 2 + 1]
                            for rb in range(NRB)]
                    # per-partition ACT scale: x2b = x2/rms as bf16, then
                    # 1-cycle/row transposes and pure-copy evictions (no
                    # scale-matrix build, no multiply on the eviction)
                    for rb in range(NRB):
                        xb = np_.tile([128, D], BF16, name=f"xb_{tag}",
                                      bufs=3)
                        nc.scalar.activation(xb[:], src_rows[rb][:], AF.Copy,
                                             scale=rscs[rb])
                        for dk in range(NDK):
                            pt = ptr_ps.tile([128, 128], BF16,
                                             name=f"ptr_{tag}", tag="ptr",
                                             bufs=4)
                            nc.tensor.transpose(
                                pt[:], xb[:, dk * 128:(dk + 1) * 128],
                                ident_b[:])
                            nc.vector.tensor_copy(
                                dst_T[:, dk, rb * 128:(rb + 1) * 128], pt[:])
                    return
                else:
                    for rb in range(NRB):
                        sq = np_.tile([128, D], src_rows[rb].dtype,
                                      name=f"sq_{tag}")
                        ss = npc.tile([128, 1], F32, name=f"ss_{tag}")
                        # alternate stats between DVE and ACT so the four
                        # row-block chains run two-wide
                        if rb % 2 == 0:
                            nc.vector.scalar_tensor_tensor(
                                sq[:], src_rows[rb][:], 1.0, src_rows[rb][:],
                                OP.mult, OP.mult, accum_out=ss[:])
                        else:
                            nc.scalar.activation(sq[:], src_rows[rb][:],
                                                 AF.Square, accum_out=ss[:])
                        nrm = npc.tile([128, 1], F32, name=f"nr_{tag}")
                        nc.scalar.activation(nrm[:], ss[:], AF.Sqrt,
                                             bias=epst[:], scale=1.0 / D)
                        rsc = npc.tile([128, 1], F32, name=f"rsc_{tag}")
                        nc.vector.reciprocal(rsc[:], nrm[:])
                        rscs.append(rsc[:])
                for rb in range(NRB):
                    psr = ptr_ps.tile([1, 128], F32, name=f"psr_{tag}",
                                      tag="psr", bufs=4)
                    nc.tensor.transpose(psr[:], rscs[rb], ident[:])
                    srow = npc.tile([1, 128], F32, name=f"srow_{tag}")
                    nc.vector.tensor_copy(srow[:], psr[:])
                    pscl = ptr_ps.tile([128, 128], F32, name=f"pscl_{tag}",
                                       tag="psr", bufs=4)
                    nc.tensor.matmul(pscl[:], ones_row[:], srow[:],
                                     start=True, stop=True)
                    scale_rb = npc.tile([128, 128], F32, name=f"scrb{rb}_{tag}")
                    nc.vector.tensor_copy(scale_rb[:], pscl[:])
                    scales.append(scale_rb)
                # rb-outer: all of row-block rb transposes as soon as its
                # source rows + scale land (earlier start vs dk-outer)
                if tr_rows is None:
                    tr_rows = src_rows
                for rb in range(NRB):
                    for dk in range(NDK):
                        pt = ptr_ps.tile([128, 128], tr_dtype,
                                         name=f"ptr_{tag}", tag="ptr", bufs=4)
                        nc.tensor.transpose(
                            pt[:], tr_rows[rb][:, dk * 128:(dk + 1) * 128],
                            tr_ident if tr_ident is not None else ident[:])
                        nc.vector.tensor_tensor(
                            dst_T[:, dk, rb * 128:(rb + 1) * 128], pt[:],
                            scales[rb][:], OP.mult)

        # ======== context phase (pools close before the MLP) ========
        with tc.tile_pool(name="ctxA", bufs=1) as ctxA, \
             tc.tile_pool(name="wctx", bufs=1) as wctx:
            # x arrives as bf16: norm1 stats tolerate it (~0.006% rms
            # error), transposes run at 1 cycle/row, and the residual x-add
            # goes through the PE as bf16; halves the head DMA
            x_loc_t = x_loc.rearrange("(rb p) d -> rb p d", p=128)
            xbf_rows = [ctxA.tile([128, D], BF16, name=f"xbr{rb}")
                        for rb in range(NRB)]
            for rb in range(NRB):
                nc.sync.dma_start(xbf_rows[rb][:], x_loc_t[rb])

            # context weights (after x: x gates the critical path)
            wf_sb = wctx.tile([128, NDK, P], BF16)
            wtoP_sb = wctx.tile([128, NDK, P], BF16)
            for kh in range(2):
                nc.sync.dma_start(wf_sb[:, kh * 4:(kh + 1) * 4, :],
                                  wf_t[:, kh * 4:(kh + 1) * 4, :])
            for kh in range(2):
                nc.sync.dma_start(wtoP_sb[:, kh * 4:(kh + 1) * 4, :],
                                  wtoP_t[:, kh * 4:(kh + 1) * 4, :])
            wfromP_sb = wctx.tile([128, NPM, D], BF16)
            br_sb = wctx.tile([128, NPM, PR], BF16)
            W2_sb = wctx.tile([PR, D], BF16)

            h1T = ctxA.tile([128, NDK, TLOC], BF16)
            fgate = ctxA.tile([128, NPM, TLOC], F32)
            t_acc = ctxA.tile([128, NPM, TLOC], BF16)
            dprods = ctxA.tile([128, NPM, TLOC], BF16)

            # ---- norm1 ----
            rmsnorm_transpose(xbf_rows, h1T, "n1",
                              tr_dtype=BF16, tr_ident=ident_b[:])

            hloc = ctxA.tile([128, NPM, TLOC], BF16)
            pay = ctxA.tile([128, 8], F32)
            # ---- context matmuls ----
            with tc.tile_pool(name="wemb_pool", bufs=4) as wemb_pool, \
                 tc.tile_pool(name="yg_pool", bufs=4) as yg_pool, \
                 tc.tile_pool(name="ctx_ps", bufs=6, space="PSUM") as ctx_ps:
                # F gate first for all m: one sigmoid table residency, and
                # the decay product comes from a DVE mult-scan (no Ln/Exp
                # table swaps on ACT)
                for m in range(NPM):
                    pq = ctx_ps.tile([128, TLOC], F32, name="pq", tag="cps")
                    for dk in range(NDK):
                        nc.tensor.matmul(pq[:],
                                         wf_sb[:, dk, m * 128:(m + 1) * 128],
                                         h1T[:, dk, :], start=(dk == 0),
                                         stop=(dk == NDK - 1))
                    nc.scalar.activation(fgate[:, m, :], pq[:], AF.Sigmoid)
                    nc.vector.tensor_tensor_scan(dprods[:, m, :],
                                                 fgate[:, m, :],
                                                 fgate[:, m, :], 1.0, OP.mult,
                                                 OP.bypass)
                    nc.vector.tensor_copy(pay[:, 4 + m:5 + m],
                                          dprods[:, m, TLOC - 1:TLOC])
                    nc.sync.dma_start(wfromP_sb[:, m, :], wfromP_t[m])

                nc.sync.dma_start(br_sb[:], br_t.rearrange("m p r -> p m r"))
                nc.sync.dma_start(W2_sb[:], W2_t[:])

                # t = sum_a gelu(U_a) + pre_path
                for m in range(NPM):
                    pp = ctx_ps.tile([128, TLOC], F32, name="pp", tag="cps")
                    for dk in range(NDK):
                        nc.tensor.matmul(pp[:],
                                         wtoP_sb[:, dk, m * 128:(m + 1) * 128],
                                         h1T[:, dk, :], start=(dk == 0),
                                         stop=(dk == NDK - 1))
                    ygs = []
                    for a in range(ALPHA):
                        wch = wemb_pool.tile([128, NDK, 128], BF16, name="wch")
                        nc.sync.dma_start(wch[:], wemb_t[a, m])
                        pu = ctx_ps.tile([128, TLOC], F32, name="pu", tag="cps")
                        for dk in range(NDK):
                            nc.tensor.matmul(pu[:], wch[:, dk, :],
                                             h1T[:, dk, :], start=(dk == 0),
                                             stop=(dk == NDK - 1))
                        yg = yg_pool.tile([128, TLOC], F32, name="yg")
                        nc.scalar.activation(yg[:], pu[:], AF.Gelu_apprx_tanh)
                        ygs.append(yg)
                    # gp sums gelu pairs in parallel with DVE
                    s01 = yg_pool.tile([128, TLOC], F32, name="s01")
                    nc.gpsimd.tensor_tensor(s01[:], ygs[0][:], ygs[1][:], OP.add)
                    s23 = yg_pool.tile([128, TLOC], F32, name="s23")
                    nc.gpsimd.tensor_tensor(s23[:], ygs[2][:], ygs[3][:], OP.add)
                    nc.vector.tensor_tensor(t_acc[:, m, :], pp[:], s01[:],
                                            OP.add)
                    nc.vector.tensor_tensor(t_acc[:, m, :], t_acc[:, m, :],
                                            s23[:], OP.add)
                    # local scan for this m (fgate[m] already computed)
                    nc.vector.tensor_tensor_scan(hloc[:, m, :], fgate[:, m, :],
                                                 t_acc[:, m, :], 0.0, OP.mult,
                                                 OP.add)
                    nc.vector.tensor_copy(pay[:, m:m + 1],
                                          hloc[:, m, TLOC - 1:TLOC])

            # mm1 weight chunks stream during the exchange window
            for hk in range(3):
                prefetch_w1(hk, at=0.045 + 0.003 * hk)

            # ---- cross-core boundary exchange ----
            with tc.tile_pool(name="agp", bufs=1) as agp, \
                 tc.tile_pool(name="agd", bufs=1, space="DRAM") as agd:
                ag_in = agd.tile([128, 8], F32)
                ag_out = agd.tile([N_CORES, 128, 8], F32,
                                  addr_space="Shared" if collective else "Local")
                nc.sync.dma_start(ag_in[:], pay[:])
                if collective:
                    nc.gpsimd.collective_compute(
                        "AllGather", OP.bypass,
                        replica_groups=[list(range(N_CORES))],
                        ins=[ag_in[:]], outs=[ag_out[:]])
                else:  # timing-sim variant: fake the gather with one
                    # broadcast DMA (DVE replicates pay 8x, single transfer)
                    pay8 = agp.tile([128, N_CORES, 8], F32, name="pay8")
                    nc.vector.tensor_copy(
                        pay8[:], pay[:].unsqueeze(1).broadcast_to(
                            [128, N_CORES, 8]))
                    nc.sync.dma_start(ag_out.rearrange("r p c -> p r c"),
                                      pay8[:])
                agbuf = agp.tile([128, N_CORES, 8], F32)
                nc.sync.dma_start(agbuf[:], ag_out.rearrange("r p c -> p r c"))
                # zero decay products of group-leading ranks (0 and 4)
                nc.vector.memset(agbuf[:, 0, 4:8], 0.0)
                nc.vector.memset(agbuf[:, 4, 4:8], 0.0)

                # y = t @ w_fromP + (h_local + CP*H_in) @ b_r @ (c_r@w_fromP).
                # The low-rank reduction sT = b_r^T @ (hloc + cpH) accumulates
                # in one [64, TLOC] psum; t parts overlap the AllGather.
                with tc.tile_pool(name="cphp", bufs=1) as cphp, \
                     tc.tile_pool(name="st_ps", bufs=1, space="PSUM") as st_ps, \
                     tc.tile_pool(name="y_ps", bufs=7, space="PSUM") as y_ps:
                    sT_ps = st_ps.tile([PR, TLOC], F32)
                    for m in range(NPM):
                        nc.tensor.matmul(
                            sT_ps[:], br_sb[:, m, :],
                            hloc[:, m, :],
                            start=(m == 0), stop=False)
                    pys = {}
                    tiles = [(rb, dn) for rb in range(NRB) for dn in range(2)]

                    def emit_t_part(rb, dn):
                        pys[rb, dn] = y_ps.tile([128, 512], F32,
                                                name="py", tag="py")
                        # x lands in PSUM via PE (idle during the gather), so
                        # the eviction is a pure copy split across DVE/ACT
                        nc.tensor.matmul(
                            pys[rb, dn][:], ident_b[:],
                            xbf_rows[rb][:, dn * 512:(dn + 1) * 512],
                            start=True, stop=False)
                        for m in range(NPM):
                            nc.tensor.matmul(
                                pys[rb, dn][:],
                                t_acc[:, m, rb * 128:(rb + 1) * 128],
                                wfromP_sb[:, m, dn * 512:(dn + 1) * 512],
                                start=False, stop=False)

                    def emit_s_part(rb, dn):
                        nc.tensor.matmul(
                            pys[rb, dn][:],
                            s_sb[:, rb * 128:(rb + 1) * 128],
                            W2_sb[:, dn * 512:(dn + 1) * 512],
                            start=False, stop=True)
                        dst = x2_rows[rb][:, dn * 512:(dn + 1) * 512]
                        if dn == 0:
                            nc.vector.tensor_copy(dst, pys[rb, dn][:])
                        else:
                            nc.scalar.activation(dst, pys[rb, dn][:], AF.Copy)

                    for rb, dn in tiles[:7]:
                        emit_t_part(rb, dn)
                    s_sb = cphp.tile([PR, TLOC], BF16)
                    # correction sT += (hin*b_r)^T @ dprod per m: H_in folds
                    # into the tiny stationary operand, no [128,512] scan
                    for m in range(NPM):
                        hall = agp.tile([128, 8], F32, name=f"hall{m}")
                        nc.vector.tensor_tensor_scan(hall[:],
                                                     agbuf[:, :, 4 + m],
                                                     agbuf[:, :, m], 0.0,
                                                     OP.mult, OP.add)
                        hm_ = agp.tile([128, 8], F32, name=f"hm{m}")
                        nc.vector.tensor_tensor(hm_[:], hall[:], hsel_t[:],
                                                OP.mult)
                        hin = agp.tile([128, 1], F32, name=f"hin{m}")
                        nc.vector.reduce_sum(hin[:], hm_[:],
                                             axis=mybir.AxisListType.X)
                        brh = agp.tile([128, PR], BF16, name=f"brh{m}")
                        nc.vector.tensor_scalar(brh[:], br_sb[:, m, :],
                                                hin[:], None, OP.mult)
                        nc.tensor.matmul(
                            sT_ps[:], brh[:], dprods[:, m, :],
                            start=False, stop=(m == NPM - 1))
                    nc.vector.tensor_copy(s_sb[:], sT_ps[:])
                    # each (rb, dn) closes and evicts as soon as its s-part
                    # lands, so norm2's per-row-block chain starts early
                    for rb, dn in tiles[:7]:
                        emit_s_part(rb, dn)
                    emit_t_part(*tiles[7])
                    emit_s_part(*tiles[7])

        # ======== norm2 + MLP (fp8e4 DoubleRow) ========
        with tc.tile_pool(name="late", bufs=1) as late:
            h2T8 = late.tile([128, NDK, TLOC], FP8)

            with tc.tile_pool(name="wfc2_pool", bufs=4) as wfc2_pool:
                # prefetch hk=0 mm2 weights before norm2
                with tc.tile_wait_until(0.058):
                    w2c_pre = wfc2_pool.tile([128, 4, 2, D], FP8, name="w2c")
                    nc.gpsimd.dma_start(w2c_pre[:], wfc2_8[0])

                rmsnorm_transpose(x2_rows, h2T8, "n2", act_stats=True)
                out_t = out_loc.rearrange("(rb p) d -> rb p d", p=128)

                def emit_mm1(hk):
                    """4 psum pair-tiles (2 oc each); gelu -> hid fp8."""
                    w1c = w1cs.pop(hk)
                    if hk + 3 < NHK:
                        prefetch_w1(hk + 3, at=0.054 if hk == 0 else None)
                    hid = hid_pool.tile([128, 4, 2, TLOC], FP8, name="hid")
                    for hbl in range(4):
                        pu = mm1_ps.tile([128, 2 * TLOC], F32, name="pu")
                        for i in range(2):
                            for kb in range(NKB):
                                nc.tensor.matmul(
                                    pu[:, i * TLOC:(i + 1) * TLOC],
                                    w1c[:, 2 * hbl + i, kb, :, :],
                                    h2T8[:, 2 * kb:2 * kb + 2, :],
                                    start=(kb == 0), stop=(kb == NKB - 1),
                                    perf_mode=DR)
                        nc.scalar.activation(
                            hid[:, hbl].rearrange("p i t -> p (i t)"), pu[:],
                            AF.Gelu_apprx_tanh, scale=1.0 / W1S)
                    return hid

                def mm2_half(pz, hid, w2c, rb, dn, start, stop):
                    for hbl in range(4):
                        nc.tensor.matmul(
                            pz[:],
                            hid[:, hbl, :, rb * 128:(rb + 1) * 128],
                            w2c[:, hbl, :, dn * 512:(dn + 1) * 512],
                            start=(start and hbl == 0),
                            stop=(stop and hbl == 3), perf_mode=DR)

                def emit_mm2(hk0, hids):
                    """accumulate 2 hk (contraction 2048 = 8 DR matmuls) into
                    psum, evict z/W2S + x2 -> x2_row in place."""
                    last = hk0 + 2 == NHK
                    for rb in range(NRB):
                        for dn in range(2):
                            pz = mm2_ps.tile([128, 512], F32, name="pz")
                            mm2_half(pz, hids[0], w2cs[hk0],
                                     rb, dn, True, False)
                            mm2_half(pz, hids[1], w2cs[hk0 + 1],
                                     rb, dn, False, True)
                            nc.vector.scalar_tensor_tensor(
                                x2_rows[rb][:, dn * 512:(dn + 1) * 512],
                                pz[:], 1.0 / W2S,
                                x2_rows[rb][:, dn * 512:(dn + 1) * 512],
                                OP.mult, OP.add)
                            if last:
                                nc.sync.dma_start(
                                    out_t[rb][:, dn * 512:(dn + 1) * 512],
                                    x2_rows[rb][:, dn * 512:(dn + 1) * 512])

                # pipeline: mm1 runs ~2 hk ahead; mm2 consumes hk pairs
                with tc.tile_pool(name="hid_pool", bufs=5) as hid_pool, \
                     tc.tile_pool(name="mm1_ps", bufs=2,
                                  space="PSUM") as mm1_ps, \
                     tc.tile_pool(name="mm2_ps", bufs=4,
                                  space="PSUM") as mm2_ps:
                    w2cs = {0: w2c_pre}
                    hids = {}
                    for hk in range(NHK):
                        if hk > 0:
                            with tc.tile_wait_until(0.058 + 0.003 * hk,
                                                    enable=hk < 4):
                                w2c = wfc2_pool.tile([128, 4, 2, D], FP8,
                                                     name="w2c")
                                nc.gpsimd.dma_start(w2c[:], wfc2_8[hk])
                            w2cs[hk] = w2c
                        hids[hk] = emit_mm1(hk)
                        if hk >= 3 and hk % 2 == 1:
                            emit_mm2(hk - 3, [hids.pop(hk - 3),
                                              hids.pop(hk - 2)])
                            w2cs.pop(hk - 3), w2cs.pop(hk - 2)
                    emit_mm2(NHK - 2, [hids.pop(NHK - 2),
                                       hids.pop(NHK - 1)])


_CACHE = {}


def _prep_weights(w_norm1, w_toP, w_emb, w_f, b_r, c_r, w_fromP, w_norm2,
                  w_fc1, w_fc2):
    import ml_dtypes
    from einops import rearrange
    E4M3 = ml_dtypes.float8_e4m3
    BF = ml_dtypes.bfloat16

    wtoP_f = (w_toP * w_norm1[:, None]).astype(BF)
    wf_f = (w_f * w_norm1[:, None]).astype(BF)
    wemb_f = (w_emb * w_norm1[:, None]).astype(BF)
    W2 = (c_r.astype(np.float64)
          @ w_fromP.astype(np.float64)).astype(np.float32).astype(BF)
    br_f = np.asarray(b_r, np.float32).astype(BF)
    wfromP_f = np.asarray(w_fromP, np.float32).astype(BF)
    wfc1_q = (w_fc1 * w_norm2[:, None] * W1S).astype(E4M3)
    wfc2_q = (w_fc2 * W2S).astype(E4M3)

    wtoP_t = np.ascontiguousarray(rearrange(wtoP_f, "(k p) n -> p k n", p=128))
    wf_t = np.ascontiguousarray(rearrange(wf_f, "(k p) n -> p k n", p=128))
    wemb_t = np.ascontiguousarray(
        rearrange(wemb_f, "(k p) (a m q) -> a m p k q", p=128, a=ALPHA, q=128))
    wfromP_t = np.ascontiguousarray(
        rearrange(wfromP_f, "(m p) d -> m p d", p=128))
    br_t = np.ascontiguousarray(rearrange(br_f, "(m p) r -> m p r", p=128))
    wfc1_8 = np.ascontiguousarray(
        rearrange(wfc1_q, "(kb i p) (hk o m) -> hk p o kb i m",
                  kb=NKB, i=2, p=128, hk=NHK, o=8, m=128))
    wfc2_8 = np.ascontiguousarray(
        rearrange(wfc2_q, "(hk h i p) d -> hk p h i d", hk=NHK, h=4, i=2,
                  p=128))
    return dict(wtoP_t=wtoP_t, wf_t=wf_t, wemb_t=wemb_t, wfromP_t=wfromP_t,
                br_t=br_t, W2_t=np.ascontiguousarray(W2), wfc1_8=wfc1_8,
                wfc2_8=wfc2_8)


def kernel(x, w_norm1, w_toP, w_emb, w_f, b_r, c_r, w_fromP, w_norm2,
           w_fc1, w_fc2):
    x = np.asarray(x, dtype=np.float32)
    if "nc" not in _CACHE:
        _CACHE["nc"] = _build_nc()
    nc = _CACHE["nc"]

    wkey = id(w_fc1)
    if _CACHE.get("wkey") != wkey:
        _CACHE["weights"] = _prep_weights(
            np.asarray(w_norm1, np.float32), np.asarray(w_toP, np.float32),
            np.asarray(w_emb, np.float32), np.asarray(w_f, np.float32),
            np.asarray(b_r, np.float32), np.asarray(c_r, np.float32),
            np.asarray(w_fromP, np.float32), np.asarray(w_norm2, np.float32),
            np.asarray(w_fc1, np.float32), np.asarray(w_fc2, np.float32))
        _CACHE["wkey"] = wkey
    weights = _CACHE["weights"]

    gsz = N_CORES // B
    in_maps = []
    for k in range(N_CORES):
        b = k // gsz
        t0 = (k % gsz) * TLOC
        hsel = np.zeros((128, 8), np.float32)
        if k % gsz != 0:
            hsel[:, k - 1] = 1.0
        import ml_dtypes
        xl = np.ascontiguousarray(x[b, t0:t0 + TLOC]).astype(ml_dtypes.bfloat16)
        in_maps.append(dict(x_loc=xl, hsel=hsel, **weights))

    res = run_bass_kernel_spmd(nc, in_maps, core_ids=list(range(N_CORES)))

    out = np.empty((B, T, D), np.float32)
    for k in range(N_CORES):
        b = k // gsz
        t0 = (k % gsz) * TLOC
        out[b, t0:t0 + TLOC] = res.results[k]["out_loc"]
    return out

